# revision 12
# baseline (speedup 1.0000x reference)
"""Trainium2 Bass kernel for nn_Cont_InfoNCE (pairwise max cross-correlation + CE loss).

Math: the reference's irfft(F1[i] * conj(F2[j]) / power) is the linear
cross-correlation of the centered rows at every lag, scaled by the positive
constant 1/(power*(T-1)).  max over lags therefore commutes with the scaling,
so dist[i,j] = max_l sum_t f1n[i,t] * f2n[j,t+l] / (T-1) with f1n = f1c/s1,
f2n = f2c/s2 (centered rows divided by their unbiased stds).

Host prep (cheap, (256,1024) numpy): center + normalize rows, quantize to
fp8e4m3, build the zero-padded Apad rows and the transposed BT layout the
device consumes directly.  Device per core: Hankel tau gather from DRAM,
dense fp8 DoubleRow matmuls over all lags (fp32 PSUM), lag-max on DVE,
scale by 1/(T-1), row-wise CE; emits one partial-loss scalar.  Sharding:
rows of zis across the 8 cores (32 each), BT replicated; host sums the 8
partials.

Dispatch: the jitted shard_map executable is AOT-compiled ONCE (via
fast_dispatch_compile, bass_effect suppressed -> C++ fast-path dispatch) and
cached; per-call work is numpy prep + one cached PJRT execute.  Going through
run_bass_kernel_spmd instead would rebuild a fresh jit closure per call and
re-run the XLA->NEFF pipeline (~0.3-0.5 s) every call.

Tiling (per core; A = prepped local zis rows, B^T = prepped zjs):
  Apad[i]    = [0^255, f1n[i], 0^257]                     (32, 1536) fp8
  Tau[t,i,u] = Apad[i, u+t]          (Hankel gather via DMA from DRAM)
  BT[t,c,j]  = f2n[j, 128c+t]                             (128, 8, 256) fp8
  for ic in 0..7, jt in 0..1, lam in 0..15:
    psum[j,ii,d'] += BT[:, 2dc:2dc+2, jtile].T @ Tau[:, ic, u0:u0+256]  (DoubleRow)
      over dc with u0 = 128*(2dc - lam + 9), valid when e0 = 2dc-lam+8 in [-1,8].
  psum[j,ii,d'] equals C[i, j, l] at lag l = 128*lam - 897 - d', covering
  every lag in [-1024, 1023] exactly once (the l = -1024 slot is identically
  0, mirroring the reference's zero-overlap k=1024 slot).
"""

import sys

if "/opt/trn_rl_repo" not in sys.path:
    sys.path.insert(0, "/opt/trn_rl_repo")

from contextlib import ExitStack

import numpy as np

import concourse.bass as bass  # noqa: F401  (keeps bass registered)
import concourse.mybir as mybir
from concourse import bacc, tile
from concourse.masks import make_identity

F32 = mybir.dt.float32
FP8 = mybir.dt.float8e4
I32 = mybir.dt.int32
X = mybir.AxisListType.X
ALU = mybir.AluOpType
ACT = mybir.ActivationFunctionType
DROW = mybir.MatmulPerfMode.DoubleRow
FP8NP = mybir.dt.np(FP8)  # ml_dtypes.float8_e4m3

M, T = 256, 1024
NCORES = 8
NLOC = M // NCORES  # 32 rows of zis per core
NIC = 4             # i-rows per i-chunk
NCHUNK = NLOC // NIC  # 8 i-chunks
TAU_U = 1408        # Hankel window extent: covers e0 in [-1, 8], +256 window
APAD = 1536         # 255 zeros + 1024 + 257 zeros


def build_nc():
    nc = bacc.Bacc("TRN2", target_bir_lowering=False)
    apad_loc = nc.dram_tensor("apad_loc", [NLOC, APAD], FP8, kind="ExternalInput")
    btj = nc.dram_tensor("btj", [128, 8, M], FP8, kind="ExternalInput")
    speeds_loc = nc.dram_tensor("speeds_loc", [NLOC, 1], I32, kind="ExternalInput")
    loss_part = nc.dram_tensor("loss_part", [1, 1], F32, kind="ExternalOutput")

    with tile.TileContext(nc) as tc, ExitStack() as ctx:
        consts = ctx.enter_context(tc.tile_pool(name="consts", bufs=1))
        prep = ctx.enter_context(tc.tile_pool(name="prep", bufs=2))
        taup = ctx.enter_context(tc.tile_pool(name="taup", bufs=3))
        ps_aux = ctx.enter_context(tc.tile_pool(name="ps_aux", bufs=2, space="PSUM"))
        ps_main = ctx.enter_context(tc.tile_pool(name="ps_main", bufs=3, space="PSUM"))

        # ---------------- constants + input loads ----------------
        ident_f32 = consts.tile([128, 128], F32)
        make_identity(nc, ident_f32)
        ones_col = consts.tile([NLOC, 1], F32)
        nc.gpsimd.memset(ones_col, 1.0)
        jidx_i = consts.tile([NLOC, M], I32)
        nc.gpsimd.iota(jidx_i, [[1, M]], base=0, channel_multiplier=0)
        jidx_f = consts.tile([NLOC, M], F32)
        nc.scalar.copy(jidx_f, jidx_i)
        sp_i = prep.tile([NLOC, 1], I32)
        nc.sync.dma_start(sp_i, speeds_loc[:, :])
        sp_f = prep.tile([NLOC, 1], F32)
        nc.scalar.copy(sp_f, sp_i)

        bt8 = consts.tile([128, 8, M], FP8)
        nc.sync.dma_start(bt8[:, :, :], btj[:, :, :])

        # ---------------- main correlation loop ------------------------------
        cmax_p = [
            consts.tile([128, 16, NLOC], F32, tag=f"cmax_{jt}", name=f"cmax_{jt}")
            for jt in range(2)
        ]
        for ic in range(NCHUNK):
            tau = taup.tile([128, NIC, TAU_U], FP8, tag="tau")
            src = apad_loc[NIC * ic:NIC * (ic + 1), 0:TAU_U]
            v = src.unsqueeze(0).broadcast_to((128, NIC, TAU_U))
            lst = v.ap
            lst[0] = [1, 128]  # Hankel: dest partition t reads Apad at +t elements
            v.ap = lst
            nc.sync.dma_start(tau[:, :, :], v)
            for jt in range(2):
                for lp in range(8):  # lambda pairs -> one 2-bank psum tile
                    ps = ps_main.tile([128, 2, NIC, 128], F32, tag="grp")
                    for q in range(2):
                        lam = 2 * lp + q
                        # valid double-chunks: e0 = 2dc - lam + 8 in [-1, 8]
                        dcs = [dc for dc in range(4) if -1 <= 2 * dc - lam + 8 <= 8]
                        for k, dc in enumerate(dcs):
                            u0 = 128 * (2 * dc - lam + 9)
                            rhs = tau[:, :, u0:u0 + 256].rearrange(
                                "p r (i d) -> p i r d", i=2
                            )
                            nc.tensor.matmul(
                                ps[:, q],
                                lhsT=bt8[:, 2 * dc:2 * dc + 2, jt * 128:(jt + 1) * 128],
                                rhs=rhs,
                                perf_mode=DROW,
                                start=(k == 0),
                                stop=(k == len(dcs) - 1),
                            )
                    nc.vector.reduce_max(
                        cmax_p[jt][:, 2 * lp:2 * lp + 2, NIC * ic:NIC * (ic + 1)],
                        ps[:, :, :, :],
                        axis=X,
                    )

        # ------------- lag-max over lambdas, scale, transpose to (i, j) -------
        dist_t = prep.tile([NLOC, M], F32)
        for jt in range(2):
            cm2 = prep.tile([128, NLOC], F32, tag="cm2")
            nc.vector.reduce_max(cm2, cmax_p[jt].rearrange("p l i -> p i l"), axis=X)
            ps_d = ps_aux.tile([NLOC, 128], F32, tag="aux")
            nc.tensor.transpose(ps_d, cm2, ident_f32)
            nc.vector.tensor_scalar_mul(
                dist_t[:, jt * 128:(jt + 1) * 128], ps_d, 1.0 / (T - 1)
            )

        # ---------------- cross-entropy (sum over local rows) -----------------
        mrow = prep.tile([NLOC, 1], F32)
        nc.vector.reduce_max(mrow, dist_t, axis=X)
        negm = prep.tile([NLOC, 1], F32)
        nc.vector.tensor_scalar_mul(negm, mrow, -1.0)
        expj = prep.tile([NLOC, M], F32)
        sumexp = prep.tile([NLOC, 1], F32)
        nc.scalar.activation(expj, dist_t, ACT.Exp, bias=negm, accum_out=sumexp)
        lse = prep.tile([NLOC, 1], F32)
        nc.scalar.activation(lse, sumexp, ACT.Ln)
        onehot = prep.tile([NLOC, M], F32)
        nc.vector.tensor_scalar(onehot, jidx_f, sp_f, None, op0=ALU.is_equal)
        junk_p = prep.tile([NLOC, M], F32)
        picked = prep.tile([NLOC, 1], F32)
        nc.vector.scalar_tensor_tensor(
            junk_p, in0=dist_t, scalar=1.0, in1=onehot, op0=ALU.mult, op1=ALU.mult, accum_out=picked
        )
        term = prep.tile([NLOC, 1], F32)
        nc.vector.tensor_add(term, lse, mrow)
        term2 = prep.tile([NLOC, 1], F32)
        nc.vector.tensor_sub(term2, term, picked)
        ps_l = ps_aux.tile([1, 1], F32, tag="aux")
        nc.tensor.matmul(ps_l, lhsT=term2, rhs=ones_col, start=True, stop=True)
        lsb = prep.tile([1, 1], F32)
        nc.vector.tensor_copy(lsb, ps_l)
        nc.sync.dma_start(loss_part[:, :], lsb)

    nc.finalize()
    return nc


# ---------------------------------------------------------------------------
# host prep: center/normalize rows, quantize fp8, build Apad + BT layouts
# ---------------------------------------------------------------------------

_APAD_BUF = np.zeros((M, APAD), dtype=FP8NP)  # zero margins persist across calls


def _norm_rows(z):
    """(z - rowmean) / unbiased rowstd, fp32 in -> fp32 out."""
    f = z - z.mean(axis=1, keepdims=True)
    ss = np.einsum("ij,ij->i", f, f)
    ss[ss == 0] = float(T - 1)
    f *= np.sqrt((T - 1) / ss)[:, None]
    return f


def _prep_inputs(zis, zjs, speeds):
    a = _norm_rows(np.asarray(zis, dtype=np.float32))
    b = _norm_rows(np.asarray(zjs, dtype=np.float32))
    _APAD_BUF[:, 255:255 + T] = a.astype(FP8NP)
    # BT[t, c, j] = b[j, 128c + t]
    bt = np.ascontiguousarray(b.astype(FP8NP).reshape(M, 8, 128).transpose(2, 1, 0))
    sp = np.ascontiguousarray(np.asarray(speeds).astype(np.int32).reshape(M, 1))
    return _APAD_BUF, bt, sp


# ---------------------------------------------------------------------------
# dispatch: AOT-compile the shard_map'd bass_exec ONCE, reuse per call
# ---------------------------------------------------------------------------

_EXEC = None  # (compiled, n_outs) or ("fallback", nc)


def _build_exec():
    import jax
    from jax.sharding import Mesh, PartitionSpec
    from concourse import bass2jax as b2j

    nc = build_nc()
    b2j.install_neuronx_cc_hook()

    dbg_name = None
    if nc.dbg_addr is not None:
        if nc.dbg_callbacks:
            raise RuntimeError("dbg_callbacks unsupported on the axon client")
        dbg_name = nc.dbg_addr.name

    partition_name = nc.partition_id_tensor.name if nc.partition_id_tensor else None

    in_names, out_names, out_avals, zero_outs = [], [], [], []
    for alloc in nc.m.functions[0].allocations:
        if not isinstance(alloc, mybir.MemoryLocationSet):
            continue
        name = alloc.memorylocations[0].name
        if alloc.kind == "ExternalInput":
            if name != partition_name:
                in_names.append(name)
        elif alloc.kind == "ExternalOutput":
            shape = tuple(alloc.tensor_shape)
            dtype = mybir.dt.np(alloc.dtype)
            out_names.append(name)
            out_avals.append(jax.core.ShapedArray(shape, dtype))
            zero_outs.append(np.zeros((NCORES * shape[0], *shape[1:]), dtype))
    n_params = len(in_names)
    n_outs = len(out_avals)
    all_in_names = list(in_names) + list(out_names)
    if partition_name is not None:
        all_in_names.append(partition_name)
    donate = tuple(range(n_params, n_params + n_outs))

    def _body(*args):
        operands = list(args)
        if partition_name is not None:
            operands.append(b2j.partition_id_tensor())
        outs = b2j._bass_exec_p.bind(
            *operands,
            out_avals=tuple(out_avals),
            in_names=tuple(all_in_names),
            out_names=tuple(out_names),
            lowering_input_output_aliases=(),
            sim_require_finite=True,
            sim_require_nnan=True,
            nc=nc,
        )
        return tuple(outs)

    devices = jax.devices()[:NCORES]
    assert len(devices) == NCORES
    mesh = Mesh(np.asarray(devices), ("core",))

    # global example args; btj is replicated (PartitionSpec()), others row-sharded
    example = {
        "apad_loc": np.zeros((M, APAD), FP8NP),
        "btj": np.zeros((128, 8, M), FP8NP),
        "speeds_loc": np.zeros((M, 1), np.int32),
    }
    if dbg_name is not None:
        example[dbg_name] = np.zeros((NCORES, 2), np.uint32)
    spec_of = {name: PartitionSpec("core") for name in in_names}
    spec_of["btj"] = PartitionSpec()
    in_specs = tuple(spec_of[name] for name in in_names) + (
        PartitionSpec("core"),
    ) * n_outs
    out_specs = (PartitionSpec("core"),) * n_outs

    from jax.experimental.shard_map import shard_map

    example_args = [example[name] for name in in_names] + zero_outs

    def _compile():
        jitted = jax.jit(
            shard_map(
                _body, mesh=mesh, in_specs=in_specs, out_specs=out_specs,
                check_rep=False,
            ),
            donate_argnums=donate,
            keep_unused=True,
        )
        return jitted.lower(*example_args).compile()

    compiled = b2j.fast_dispatch_compile(_compile)

    # Pre-warm: run the executable twice with dummy inputs and fetch the
    # result, flushing lazy dispatch/transfer-path init so the first real
    # call pays only numpy prep + one wire round trip.
    for _ in range(2):
        warm = [example[name] for name in in_names] + [
            np.zeros_like(z) for z in zero_outs
        ]
        np.asarray(compiled(*warm)[0])

    return compiled, in_names, zero_outs, dbg_name


def _get_exec():
    global _EXEC
    if _EXEC is None:
        try:
            _EXEC = _build_exec()
        except Exception:
            _EXEC = ("fallback", build_nc())
    return _EXEC


def _run_fallback(nc, apad, bt, sp):
    from concourse.bass_utils import run_bass_kernel_spmd

    in_maps = [
        {
            "apad_loc": np.ascontiguousarray(apad[c * NLOC:(c + 1) * NLOC]),
            "btj": bt,
            "speeds_loc": np.ascontiguousarray(sp[c * NLOC:(c + 1) * NLOC]),
        }
        for c in range(NCORES)
    ]
    res = run_bass_kernel_spmd(nc, in_maps, core_ids=list(range(NCORES)))
    return np.float32(sum(float(r["loss_part"][0, 0]) for r in res.results))


LAST_RESULT = None  # kept for test.py compatibility (no trace under axon)
_DBG_ZERO = np.zeros((NCORES, 2), np.uint32)


def run(zis, zjs, speeds, trace=False):
    exc = _get_exec()
    apad, bt, sp = _prep_inputs(zis, zjs, speeds)
    if isinstance(exc[0], str):  # ("fallback", nc)
        return _run_fallback(exc[1], apad, bt, sp)
    compiled, in_names, zero_outs, dbg_name = exc
    vals = {"apad_loc": apad, "btj": bt, "speeds_loc": sp}
    if dbg_name is not None:
        vals[dbg_name] = _DBG_ZERO
    args = [vals[name] for name in in_names] + zero_outs
    # Transient device errors (NRT_EXEC_UNIT_UNRECOVERABLE etc.) usually
    # clear on re-execute; retry before dropping to the slow spmd path.
    for attempt in range(3):
        try:
            outs = compiled(*args)
            return np.float32(np.asarray(outs[0]).sum())
        except Exception:
            if attempt == 2:
                break
            import time

            time.sleep(0.2)
    return _run_fallback(build_nc(), apad, bt, sp)


def kernel(zis, zjs, speeds):
    return run(zis, zjs, speeds, trace=False)


# revision 14
# speedup vs baseline: 1.0140x; 1.0140x over previous
"""Trainium2 Bass kernel for nn_Cont_InfoNCE (pairwise max cross-correlation + CE loss).

Math: the reference's irfft(F1[i] * conj(F2[j]) / power) is the linear
cross-correlation of the centered rows at every lag, scaled by the positive
constant 1/(power*(T-1)).  max over lags therefore commutes with the scaling,
so dist[i,j] = max_l sum_t f1n[i,t] * f2n[j,t+l] / (T-1) with f1n = f1c/s1,
f2n = f2c/s2 (centered rows divided by their unbiased stds).

Host prep (cheap, (256,1024) numpy): center + normalize rows, quantize to
fp8e4m3, build the zero-padded Apad rows and the transposed BT layout the
device consumes directly.  Device per core: Hankel tau gather from DRAM,
dense fp8 DoubleRow matmuls over all lags (fp32 PSUM), lag-max on DVE,
scale by 1/(T-1), row-wise CE; emits one partial-loss scalar.  Sharding:
rows of zis across the 8 cores (32 each), BT replicated; host sums the 8
partials.

Dispatch: the jitted shard_map executable is AOT-compiled ONCE (via
fast_dispatch_compile, bass_effect suppressed -> C++ fast-path dispatch) and
cached; per-call work is numpy prep + one cached PJRT execute.  Going through
run_bass_kernel_spmd instead would rebuild a fresh jit closure per call and
re-run the XLA->NEFF pipeline (~0.3-0.5 s) every call.

Tiling (per core; A = prepped local zis rows, B^T = prepped zjs):
  Apad[i]    = [0^255, f1n[i], 0^257]                     (32, 1536) fp8
  Tau[t,i,u] = Apad[i, u+t]          (Hankel gather via DMA from DRAM)
  BT[t,c,j]  = f2n[j, 128c+t]                             (128, 8, 256) fp8
  for ic in 0..7, jt in 0..1, lam in 0..15:
    psum[j,ii,d'] += BT[:, 2dc:2dc+2, jtile].T @ Tau[:, ic, u0:u0+256]  (DoubleRow)
      over dc with u0 = 128*(2dc - lam + 9), valid when e0 = 2dc-lam+8 in [-1,8].
  psum[j,ii,d'] equals C[i, j, l] at lag l = 128*lam - 897 - d', covering
  every lag in [-1024, 1023] exactly once (the l = -1024 slot is identically
  0, mirroring the reference's zero-overlap k=1024 slot).
"""

import sys

if "/opt/trn_rl_repo" not in sys.path:
    sys.path.insert(0, "/opt/trn_rl_repo")

from contextlib import ExitStack

import numpy as np

import concourse.bass as bass  # noqa: F401  (keeps bass registered)
import concourse.mybir as mybir
from concourse import bacc, tile
from concourse.masks import make_identity

F32 = mybir.dt.float32
FP8 = mybir.dt.float8e4
I32 = mybir.dt.int32
X = mybir.AxisListType.X
ALU = mybir.AluOpType
ACT = mybir.ActivationFunctionType
DROW = mybir.MatmulPerfMode.DoubleRow
FP8NP = mybir.dt.np(FP8)  # ml_dtypes.float8_e4m3

M, T = 256, 1024
NCORES = 8
NLOC = M // NCORES  # 32 rows of zis per core
NIC = 4             # i-rows per i-chunk
NCHUNK = NLOC // NIC  # 8 i-chunks
TAU_U = 1408        # Hankel window extent: covers e0 in [-1, 8], +256 window
APAD = 1536         # 255 zeros + 1024 + 257 zeros


def build_nc():
    nc = bacc.Bacc("TRN2", target_bir_lowering=False)
    apad_loc = nc.dram_tensor("apad_loc", [NLOC, APAD], FP8, kind="ExternalInput")
    btj = nc.dram_tensor("btj", [128, 8, M], FP8, kind="ExternalInput")
    speeds_loc = nc.dram_tensor("speeds_loc", [NLOC, 1], I32, kind="ExternalInput")
    loss_part = nc.dram_tensor("loss_part", [1, 1], F32, kind="ExternalOutput")

    with tile.TileContext(nc) as tc, ExitStack() as ctx:
        consts = ctx.enter_context(tc.tile_pool(name="consts", bufs=1))
        prep = ctx.enter_context(tc.tile_pool(name="prep", bufs=2))
        taup = ctx.enter_context(tc.tile_pool(name="taup", bufs=3))
        ps_aux = ctx.enter_context(tc.tile_pool(name="ps_aux", bufs=2, space="PSUM"))
        ps_main = ctx.enter_context(tc.tile_pool(name="ps_main", bufs=3, space="PSUM"))

        # ---------------- constants + input loads ----------------
        ident_f32 = consts.tile([128, 128], F32)
        make_identity(nc, ident_f32)
        ones_col = consts.tile([NLOC, 1], F32)
        nc.gpsimd.memset(ones_col, 1.0)
        jidx_i = consts.tile([NLOC, M], I32)
        nc.gpsimd.iota(jidx_i, [[1, M]], base=0, channel_multiplier=0)
        jidx_f = consts.tile([NLOC, M], F32)
        nc.scalar.copy(jidx_f, jidx_i)
        sp_i = prep.tile([NLOC, 1], I32)
        nc.sync.dma_start(sp_i, speeds_loc[:, :])
        sp_f = prep.tile([NLOC, 1], F32)
        nc.scalar.copy(sp_f, sp_i)

        bt8 = consts.tile([128, 8, M], FP8)
        nc.sync.dma_start(bt8[:, :, :], btj[:, :, :])

        # ---------------- main correlation loop ------------------------------
        cmax_p = [
            consts.tile([128, 16, NLOC], F32, tag=f"cmax_{jt}", name=f"cmax_{jt}")
            for jt in range(2)
        ]
        for ic in range(NCHUNK):
            tau = taup.tile([128, NIC, TAU_U], FP8, tag="tau")
            src = apad_loc[NIC * ic:NIC * (ic + 1), 0:TAU_U]
            v = src.unsqueeze(0).broadcast_to((128, NIC, TAU_U))
            lst = v.ap
            lst[0] = [1, 128]  # Hankel: dest partition t reads Apad at +t elements
            v.ap = lst
            nc.sync.dma_start(tau[:, :, :], v)
            for jt in range(2):
                for lp in range(8):  # lambda pairs -> one 2-bank psum tile
                    ps = ps_main.tile([128, 2, NIC, 128], F32, tag="grp")
                    for q in range(2):
                        lam = 2 * lp + q
                        # valid double-chunks: e0 = 2dc - lam + 8 in [-1, 8]
                        dcs = [dc for dc in range(4) if -1 <= 2 * dc - lam + 8 <= 8]
                        for k, dc in enumerate(dcs):
                            u0 = 128 * (2 * dc - lam + 9)
                            rhs = tau[:, :, u0:u0 + 256].rearrange(
                                "p r (i d) -> p i r d", i=2
                            )
                            nc.tensor.matmul(
                                ps[:, q],
                                lhsT=bt8[:, 2 * dc:2 * dc + 2, jt * 128:(jt + 1) * 128],
                                rhs=rhs,
                                perf_mode=DROW,
                                start=(k == 0),
                                stop=(k == len(dcs) - 1),
                            )
                    nc.vector.reduce_max(
                        cmax_p[jt][:, 2 * lp:2 * lp + 2, NIC * ic:NIC * (ic + 1)],
                        ps[:, :, :, :],
                        axis=X,
                    )

        # ------------- lag-max over lambdas, scale, transpose to (i, j) -------
        dist_t = prep.tile([NLOC, M], F32)
        for jt in range(2):
            cm2 = prep.tile([128, NLOC], F32, tag="cm2")
            nc.vector.reduce_max(cm2, cmax_p[jt].rearrange("p l i -> p i l"), axis=X)
            ps_d = ps_aux.tile([NLOC, 128], F32, tag="aux")
            nc.tensor.transpose(ps_d, cm2, ident_f32)
            nc.vector.tensor_scalar_mul(
                dist_t[:, jt * 128:(jt + 1) * 128], ps_d, 1.0 / (T - 1)
            )

        # ---------------- cross-entropy (sum over local rows) -----------------
        mrow = prep.tile([NLOC, 1], F32)
        nc.vector.reduce_max(mrow, dist_t, axis=X)
        negm = prep.tile([NLOC, 1], F32)
        nc.vector.tensor_scalar_mul(negm, mrow, -1.0)
        expj = prep.tile([NLOC, M], F32)
        sumexp = prep.tile([NLOC, 1], F32)
        nc.scalar.activation(expj, dist_t, ACT.Exp, bias=negm, accum_out=sumexp)
        lse = prep.tile([NLOC, 1], F32)
        nc.scalar.activation(lse, sumexp, ACT.Ln)
        onehot = prep.tile([NLOC, M], F32)
        nc.vector.tensor_scalar(onehot, jidx_f, sp_f, None, op0=ALU.is_equal)
        junk_p = prep.tile([NLOC, M], F32)
        picked = prep.tile([NLOC, 1], F32)
        nc.vector.scalar_tensor_tensor(
            junk_p, in0=dist_t, scalar=1.0, in1=onehot, op0=ALU.mult, op1=ALU.mult, accum_out=picked
        )
        term = prep.tile([NLOC, 1], F32)
        nc.vector.tensor_add(term, lse, mrow)
        term2 = prep.tile([NLOC, 1], F32)
        nc.vector.tensor_sub(term2, term, picked)
        ps_l = ps_aux.tile([1, 1], F32, tag="aux")
        nc.tensor.matmul(ps_l, lhsT=term2, rhs=ones_col, start=True, stop=True)
        lsb = prep.tile([1, 1], F32)
        nc.vector.tensor_copy(lsb, ps_l)
        nc.sync.dma_start(loss_part[:, :], lsb)

    nc.finalize()
    return nc


# ---------------------------------------------------------------------------
# host prep: center/normalize rows, quantize fp8, build Apad + BT layouts
# ---------------------------------------------------------------------------

_APAD_BUF = np.zeros((M, APAD), dtype=FP8NP)  # zero margins persist across calls


def _norm_rows(z):
    """(z - rowmean) / unbiased rowstd, fp32 in -> fp32 out."""
    f = z - z.mean(axis=1, keepdims=True)
    ss = np.einsum("ij,ij->i", f, f)
    ss[ss == 0] = float(T - 1)
    f *= np.sqrt((T - 1) / ss)[:, None]
    return f


def _prep_inputs(zis, zjs, speeds):
    a = _norm_rows(np.asarray(zis, dtype=np.float32))
    b = _norm_rows(np.asarray(zjs, dtype=np.float32))
    _APAD_BUF[:, 255:255 + T] = a.astype(FP8NP)
    # BT[t, c, j] = b[j, 128c + t]
    bt = np.ascontiguousarray(b.astype(FP8NP).reshape(M, 8, 128).transpose(2, 1, 0))
    sp = np.ascontiguousarray(np.asarray(speeds).astype(np.int32).reshape(M, 1))
    return _APAD_BUF, bt, sp


# ---------------------------------------------------------------------------
# dispatch: AOT-compile the shard_map'd bass_exec ONCE, reuse per call
# ---------------------------------------------------------------------------

_EXEC = None  # (compiled, n_outs) or ("fallback", nc)


def _build_exec():
    import jax
    from jax.sharding import Mesh, PartitionSpec
    from concourse import bass2jax as b2j

    nc = build_nc()
    b2j.install_neuronx_cc_hook()

    dbg_name = None
    if nc.dbg_addr is not None:
        if nc.dbg_callbacks:
            raise RuntimeError("dbg_callbacks unsupported on the axon client")
        dbg_name = nc.dbg_addr.name

    partition_name = nc.partition_id_tensor.name if nc.partition_id_tensor else None

    in_names, out_names, out_avals, zero_outs = [], [], [], []
    for alloc in nc.m.functions[0].allocations:
        if not isinstance(alloc, mybir.MemoryLocationSet):
            continue
        name = alloc.memorylocations[0].name
        if alloc.kind == "ExternalInput":
            if name != partition_name:
                in_names.append(name)
        elif alloc.kind == "ExternalOutput":
            shape = tuple(alloc.tensor_shape)
            dtype = mybir.dt.np(alloc.dtype)
            out_names.append(name)
            out_avals.append(jax.core.ShapedArray(shape, dtype))
            zero_outs.append(np.zeros((NCORES * shape[0], *shape[1:]), dtype))
    n_params = len(in_names)
    n_outs = len(out_avals)
    all_in_names = list(in_names) + list(out_names)
    if partition_name is not None:
        all_in_names.append(partition_name)
    donate = tuple(range(n_params, n_params + n_outs))

    def _body(*args):
        operands = list(args)
        if partition_name is not None:
            operands.append(b2j.partition_id_tensor())
        outs = b2j._bass_exec_p.bind(
            *operands,
            out_avals=tuple(out_avals),
            in_names=tuple(all_in_names),
            out_names=tuple(out_names),
            lowering_input_output_aliases=(),
            sim_require_finite=True,
            sim_require_nnan=True,
            nc=nc,
        )
        return tuple(outs)

    devices = jax.devices()[:NCORES]
    assert len(devices) == NCORES
    mesh = Mesh(np.asarray(devices), ("core",))

    # global example args; btj is replicated (PartitionSpec()), others row-sharded
    example = {
        "apad_loc": np.zeros((M, APAD), FP8NP),
        "btj": np.zeros((128, 8, M), FP8NP),
        "speeds_loc": np.zeros((M, 1), np.int32),
    }
    if dbg_name is not None:
        example[dbg_name] = np.zeros((NCORES, 2), np.uint32)
    spec_of = {name: PartitionSpec("core") for name in in_names}
    spec_of["btj"] = PartitionSpec()
    in_specs = tuple(spec_of[name] for name in in_names) + (
        PartitionSpec("core"),
    ) * n_outs
    out_specs = (PartitionSpec("core"),) * n_outs

    from jax.experimental.shard_map import shard_map

    example_args = [example[name] for name in in_names] + zero_outs

    def _compile():
        jitted = jax.jit(
            shard_map(
                _body, mesh=mesh, in_specs=in_specs, out_specs=out_specs,
                check_rep=False,
            ),
            donate_argnums=donate,
            keep_unused=True,
        )
        return jitted.lower(*example_args).compile()

    compiled = b2j.fast_dispatch_compile(_compile)

    # Pre-warm: run the executable twice with dummy inputs and fetch the
    # result, flushing lazy dispatch/transfer-path init so the first real
    # call pays only numpy prep + one wire round trip.
    for _ in range(2):
        warm = [example[name] for name in in_names] + [
            np.zeros_like(z) for z in zero_outs
        ]
        np.asarray(compiled(*warm)[0])

    return compiled, in_names, zero_outs, dbg_name


def _get_exec():
    global _EXEC
    if _EXEC is None:
        try:
            _EXEC = _build_exec()
        except Exception:
            _EXEC = ("fallback", build_nc())
    return _EXEC


def _run_fallback(nc, apad, bt, sp):
    from concourse.bass_utils import run_bass_kernel_spmd

    in_maps = [
        {
            "apad_loc": np.ascontiguousarray(apad[c * NLOC:(c + 1) * NLOC]),
            "btj": bt,
            "speeds_loc": np.ascontiguousarray(sp[c * NLOC:(c + 1) * NLOC]),
        }
        for c in range(NCORES)
    ]
    res = run_bass_kernel_spmd(nc, in_maps, core_ids=list(range(NCORES)))
    return np.float32(sum(float(r["loss_part"][0, 0]) for r in res.results))


LAST_RESULT = None  # kept for test.py compatibility (no trace under axon)
_DBG_ZERO = np.zeros((NCORES, 2), np.uint32)


def run(zis, zjs, speeds, trace=False):
    exc = _get_exec()
    apad, bt, sp = _prep_inputs(zis, zjs, speeds)
    if isinstance(exc[0], str):  # ("fallback", nc)
        return _run_fallback(exc[1], apad, bt, sp)
    compiled, in_names, zero_outs, dbg_name = exc
    vals = {"apad_loc": apad, "btj": bt, "speeds_loc": sp}
    if dbg_name is not None:
        vals[dbg_name] = _DBG_ZERO
    args = [vals[name] for name in in_names] + zero_outs
    # Transient device errors (NRT_EXEC_UNIT_UNRECOVERABLE etc.) usually
    # clear on re-execute; retry before dropping to the slow spmd path.
    for attempt in range(3):
        try:
            outs = compiled(*args)
            return np.float32(np.asarray(outs[0]).sum())
        except Exception:
            if attempt == 2:
                break
            import time

            time.sleep(0.2)
    return _run_fallback(build_nc(), apad, bt, sp)


def kernel(zis, zjs, speeds):
    return run(zis, zjs, speeds, trace=False)
